# revision 20
# baseline (speedup 1.0000x reference)
"""Trainium2 Bass kernel for nn_MoEDiscriminator (8 experts, MLP 64->256->256->1).

Strategy (data-parallel over 8 NeuronCores, all-bf16 matmuls):
- st [65536, 64] sharded along batch: 8192 rows/core; weights replicated.
- Activations live as [feature_on_partitions, batch_on_free] SBUF tiles.
- PE-array tile packing (tile_position) exploits the small dims:
    L1 (K=64):  two experts run CONCURRENTLY in row groups 0-1 / 2-3
                (st is duplicated on partitions 64:128).  ~164ns / pair slot.
    L2 (M=256): each 128-col output block is split into 4 M=32 col-group
                tiles sharing one h1 moving stream.       ~108ns / quad slot.
    L3 (M=1):   4 experts' W3 columns run concurrently in 4 col groups,
                accumulating into one PSUM bank.          ~108ns / quad slot.
- PSUM->SBUF evictions (relu+bias) are the wall (~150-160us/core across
  ACT+DVE at 1 elem/lane/cycle).  L2 evictions are widened to [128,1024]
  (two banks, batch-tile pairs) to amortize the fixed access latency; the
  engine split is tuned so ACT (1.2GHz) takes the wide tiles.
- d stays [128, 512] per tile in PSUM with experts on rows 32*(c%4)+16*(c/4);
  a full-width copy lands it in SBUF and 8 row-DMAs write the output.
- b3 is added on the host.
"""

import sys

sys.path.insert(0, "/opt/trn_rl_repo")
from contextlib import ExitStack

import numpy as np

import concourse.bass as bass
import concourse.tile as tile
from concourse import bacc, mybir
from concourse.bass import ts
from concourse.bass_utils import run_bass_kernel_spmd

P = 128
C = 8            # experts
DS = 64          # input feature dim
H = 256          # hidden width
B = 65536        # full batch
NCORES = 8
NB = B // NCORES  # 8192 rows per core
BT = 512         # batch tile (free dim of matmuls)
NT = NB // BT    # 16 batch tiles -> 8 tile-pairs
ST_CHUNKS = (512, 1536, 2048, 4096)   # graduated st chunk widths (bf16 cols)
L1_ACT_PER8 = 1    # of every 8 narrow L1 evictions, this many go to ACT
WIDE_ACT = True    # wide L2 evictions on ACT (else DVE)
WIDE_EVICT = False # pair L2 evictions across sub-tiles ([128,1024], 2 banks)
NARROW_ACT_MOD = (2, 1)  # when WIDE_EVICT=False: (den, num) ACT share
L2_QUADS = False   # L2 as 4x M=32 col-group tiles (bf16) vs serial M=128 f32r

f32 = mybir.dt.float32
bf16 = mybir.dt.bfloat16
AF = mybir.ActivationFunctionType
ALU = mybir.AluOpType

_NC_CACHE = {}


def _rowmap(c):
    return 32 * (c % 4) + 16 * (c // 4)


def _build_nc(repeats=1):
    key = (repeats, ST_CHUNKS, L1_ACT_PER8, WIDE_ACT, WIDE_EVICT,
           NARROW_ACT_MOD, L2_QUADS)
    if key in _NC_CACHE:
        return _NC_CACHE[key]
    nc = bacc.Bacc("TRN2", target_bir_lowering=False, debug=False,
                   num_devices=NCORES)
    st_d = nc.dram_tensor("st", [P, NB], bf16, kind="ExternalInput").ap()
    # boot: w1 pair0 (both M-blocks); biases ride in their own f32 tensor
    boot_d = nc.dram_tensor("boot", [P, 2 * P], bf16,
                            kind="ExternalInput").ap()
    bias_d = nc.dram_tensor("bias", [P, 32], f32, kind="ExternalInput").ap()
    w1_d = nc.dram_tensor("w1", [4, 2, P, P], bf16, kind="ExternalInput").ap()
    w2_dt = bf16 if L2_QUADS else mybir.dt.float32r
    w2_d = nc.dram_tensor("w2", [C, 2, 2, P, P], w2_dt,
                          kind="ExternalInput").ap()
    w3_d = nc.dram_tensor("w3", [C, 2, P, 32], bf16, kind="ExternalInput").ap()
    d_d = nc.dram_tensor("d", [P, NB], bf16, kind="ExternalOutput").ap()

    with tile.TileContext(nc) as tc, ExitStack() as ctx:
        const = ctx.enter_context(tc.tile_pool(name="const", bufs=2))
        work1 = ctx.enter_context(tc.tile_pool(name="work1", bufs=14))
        work2 = ctx.enter_context(tc.tile_pool(name="work2", bufs=17))
        psumA = ctx.enter_context(tc.tile_pool(
            name="psumA", bufs=2 if WIDE_EVICT else 3, space="PSUM"))
        psumB = ctx.enter_context(tc.tile_pool(
            name="psumB", bufs=2 if WIDE_EVICT else 3, space="PSUM"))
        psumD = ctx.enter_context(tc.tile_pool(name="psumD", bufs=1,
                                               space="PSUM"))

        def body():
            # --- boot DMAs (small first so compute starts early) ---
            boot_sb = const.tile([P, 2 * P], bf16)
            nc.sync.dma_start(boot_sb[:], boot_d)
            bias_sb = const.tile([P, 32], f32)
            nc.sync.dma_start(bias_sb[:], bias_d)
            b1_sb = bias_sb[:, 0:16]
            b2_sb = bias_sb[:, 16:32]

            st_sb, st_off = [], []
            off = 0
            for i, cols in enumerate(ST_CHUNKS):
                t_ = const.tile([P, cols], bf16, name=f"st_sb{i}")
                st_sb.append(t_)
                st_off.append(off)
                off += cols
            assert off == NB
            nc.sync.dma_start(st_sb[0][:], st_d[:, 0:ST_CHUNKS[0]])

            w2_sb = const.tile([P, C, 2, 2, P], w2_dt)
            nc.sync.dma_start(w2_sb[:, 0:2],
                              w2_d[0:2].rearrange("c k j p f -> p c k j f"))
            w1_sb = const.tile([P, 4, 2, P], bf16)
            nc.sync.dma_start(w1_sb[:, 1:4],
                              w1_d[1:4].rearrange("a m p f -> p a m f"))
            nc.sync.dma_start(st_sb[1][:],
                              st_d[:, st_off[1]:st_off[1] + ST_CHUNKS[1]])
            w3_sb = const.tile([P, C, 2, 32], bf16)
            nc.sync.dma_start(w3_sb[:], w3_d.rearrange("c k p f -> p c k f"))
            nc.sync.dma_start(w2_sb[:, 2:5],
                              w2_d[2:5].rearrange("c k j p f -> p c k j f"))
            nc.sync.dma_start(st_sb[2][:],
                              st_d[:, st_off[2]:st_off[2] + ST_CHUNKS[2]])
            nc.sync.dma_start(w2_sb[:, 5:8],
                              w2_d[5:8].rearrange("c k j p f -> p c k j f"))
            nc.sync.dma_start(st_sb[3][:],
                              st_d[:, st_off[3]:st_off[3] + ST_CHUNKS[3]])

            d_wide = const.tile([P, NB], bf16)

            def w1_ap(p, m):
                if p == 0:
                    return boot_sb[:, m * P:(m + 1) * P]
                return w1_sb[:, p, m, :]

            def st_slice(t, lo, hi):
                col = t * BT
                for i, o in enumerate(st_off):
                    if o <= col < o + ST_CHUNKS[i]:
                        return st_sb[i][lo:hi, col - o:col - o + BT]
                raise AssertionError

            ncnt = [0]

            def evict_narrow(dst, src, bias_col):
                if WIDE_EVICT:
                    use_act = (ncnt[0] % 8) < L1_ACT_PER8
                else:
                    use_act = (ncnt[0] % NARROW_ACT_MOD[0]) < NARROW_ACT_MOD[1]
                ncnt[0] += 1
                if use_act:
                    nc.scalar.activation(dst, src, AF.Relu, bias=bias_col)
                else:
                    nc.vector.tensor_scalar(dst, src, bias_col, 0.0,
                                            ALU.add, ALU.max)

            wcnt = [0]

            def evict_wide(dst, src, bias_col):
                use_act = (wcnt[0] % 2) == 0
                wcnt[0] += 1
                if use_act:
                    nc.scalar.activation(dst, src, AF.Relu, bias=bias_col)
                else:
                    nc.vector.tensor_scalar(dst, src, bias_col, 0.0,
                                            ALU.add, ALU.max)

            h1s, h2s = {}, {}

            def l1_slot(T, p, s, m):
                """Two experts (2p, 2p+1) concurrently: K=64 row tiles."""
                t = 2 * T + s
                pA = [psumA.tile([P, BT], f32, tag="pA", name=f"pA{e}")
                      for e in range(2)]
                nc.tensor.matmul(pA[0][:], w1_ap(p, m)[0:64, :],
                                 st_slice(t, 0, 64), start=True, stop=True,
                                 tile_position=(0, 0))
                nc.tensor.matmul(pA[1][:], w1_ap(p, m)[64:128, :],
                                 st_slice(t, 64, 128), start=True, stop=True,
                                 tile_position=(64, 0))
                for e in range(2):
                    c = 2 * p + e
                    h1 = work1.tile([P, BT], bf16 if L2_QUADS
                                    else mybir.dt.float32r,
                                    tag="h1", name=f"h1_{e}")
                    evict_narrow(h1[:], pA[e][:], b1_sb[:, 2 * c + m:2 * c + m + 1])
                    h1s[(s, c, m)] = h1

            def l2_expert_half(T, c, half):
                """One expert's 128 h2 features (M-block `half`) for both
                sub-tiles: wide [128,1024] psum, 4 serial M=128 matmuls."""
                h2 = work2.tile([P, 2 * BT], bf16, tag="h2", name="h2")
                if WIDE_EVICT:
                    w = psumB.tile([P, 2 * BT], f32, tag="pB", name="pB")
                    for s in range(2):
                        for chunk in range(2):
                            nc.tensor.matmul(
                                w[:, ts(s, BT)],
                                w2_sb[:, c, chunk, half, :],
                                h1s[(s, c, chunk)][:],
                                start=(chunk == 0), stop=(chunk == 1),
                                skip_group_check=True)
                    evict_wide(h2[:], w[:],
                               b2_sb[:, 2 * c + half:2 * c + half + 1])
                else:
                    for s in range(2):
                        w = psumB.tile([P, BT], f32, tag="pB", name="pB")
                        for chunk in range(2):
                            if L2_QUADS:
                                for j in range(4):
                                    nc.tensor.matmul(
                                        w[32 * j:32 * j + 32, :],
                                        w2_sb[:, c, chunk, half,
                                              32 * j:32 * j + 32],
                                        h1s[(s, c, chunk)][:],
                                        start=(chunk == 0), stop=(chunk == 1),
                                        tile_position=(0, 32 * j),
                                        skip_group_check=True)
                            else:
                                nc.tensor.matmul(
                                    w[:], w2_sb[:, c, chunk, half, :],
                                    h1s[(s, c, chunk)][:],
                                    start=(chunk == 0), stop=(chunk == 1),
                                    skip_group_check=True)
                        evict_narrow(h2[:, ts(s, BT)], w[:],
                                     b2_sb[:, 2 * c + half:2 * c + half + 1])
                h2s[(c, half)] = h2

            pDs = {}

            def l3_quads(T, clo, start, stop):
                if start:
                    pDs[0] = psumD.tile([P, 2 * BT], f32, tag="pD", name="pD")
                pD = pDs[0]
                for s in range(2):
                    for chunk in range(2):
                        for c in range(clo, clo + 4):
                            nc.tensor.matmul(
                                pD[32 * (c % 4):32 * (c % 4) + 32, ts(s, BT)],
                                w3_sb[:, c, chunk, :],
                                h2s[(c, chunk)][:, ts(s, BT)],
                                start=(start and chunk == 0),
                                stop=(stop and chunk == 1),
                                tile_position=(0, 32 * (c % 4)),
                                skip_group_check=True)
                if stop:
                    cols = slice(T * 2 * BT, (T + 1) * 2 * BT)
                    if T % 2 == 0:
                        nc.scalar.copy(d_wide[:, cols], pDs.pop(0)[:])
                    else:
                        nc.vector.tensor_copy(d_wide[:, cols], pDs.pop(0)[:])

            # Flat software pipeline: L1 of item i interleaves with L2 of
            # item i-1 so PE always has streamable work while evictions of
            # the newest L1 psum tiles are in flight.
            NITEMS = (NT // 2) * 4

            def emit_step(i):
                l1 = None
                if i < NITEMS:
                    T, p = divmod(i, 4)
                    l1 = [(T, p, s, m) for s in range(2) for m in range(2)]
                l2 = None
                if i > 0:
                    T2, p2 = divmod(i - 1, 4)
                    l2 = [(T2, 2 * p2 + e, half)
                          for e in range(2) for half in range(2)]
                for k in range(4):
                    if l1 is not None:
                        l1_slot(*l1[k])
                    if l2 is not None:
                        l2_expert_half(*l2[k])
                if l2 is not None:
                    if p2 == 1:
                        l3_quads(T2, 0, start=True, stop=False)
                    elif p2 == 3:
                        l3_quads(T2, 4, start=False, stop=True)
                        h2s.clear()
                        nc.sync.dma_start(
                            d_d[:, T2 * 2 * BT:(T2 + 1) * 2 * BT],
                            d_wide[:, T2 * 2 * BT:(T2 + 1) * 2 * BT])

            for i in range(NITEMS + 1):
                emit_step(i)

        for _rep in range(repeats):
            body()

    nc.compile()
    _NC_CACHE[key] = nc
    return nc


def _prep_weights(W1, b1, W2, b2, W3):
    import ml_dtypes
    bf = ml_dtypes.bfloat16
    w1p = np.zeros((4, 2, P, P), np.float32)
    for p in range(4):
        for m in range(2):
            w1p[p, m, 0:DS, :] = W1[2 * p][:, m * P:(m + 1) * P]
            w1p[p, m, DS:2 * DS, :] = W1[2 * p + 1][:, m * P:(m + 1) * P]
    w2q = np.ascontiguousarray(
        W2.reshape(C, 2, P, 2, P).transpose(0, 1, 3, 2, 4)).astype(np.float32)
    if L2_QUADS:
        w2q = w2q.astype(bf)
    # w2q[c,k,j] = W2[c, k*128:(k+1)*128, j*128:(j+1)*128]
    w3q = np.zeros((C, 2, P, 32), np.float32)
    for c in range(C):
        for k in range(2):
            w3q[c, k, :, 16 * (c // 4)] = W3[c, k * P:(k + 1) * P, 0]
    b1h = np.ascontiguousarray(b1.reshape(C * 2, P).T)  # [128, 16] f32
    b2h = np.ascontiguousarray(b2.reshape(C * 2, P).T)
    boot = np.concatenate([w1p[0, 0].astype(bf), w1p[0, 1].astype(bf)],
                          axis=1)
    biases = np.concatenate([b1h, b2h], axis=1).astype(np.float32)
    return (w1p.astype(bf), w2q, w3q.astype(bf),
            np.ascontiguousarray(boot), np.ascontiguousarray(biases))


def _make_in_maps(st, W1, b1, W2, b2, W3):
    import ml_dtypes
    bf = ml_dtypes.bfloat16
    w1p, w2q, w3q, boot, biases = _prep_weights(W1, b1, W2, b2, W3)
    in_maps = []
    for core in range(NCORES):
        shard = st[core * NB:(core + 1) * NB]             # [8192, 64]
        stT = np.ascontiguousarray(
            np.concatenate([shard.T, shard.T], axis=0).astype(bf))
        in_maps.append({"st": stT, "boot": boot, "bias": biases, "w1": w1p,
                        "w2": w2q, "w3": w3q})
    return in_maps


class _SpmdExec:
    """Reusable jitted shard_map executor for a compiled Bass module."""

    def __init__(self, nc, n_cores):
        import jax
        from jax.sharding import Mesh, PartitionSpec
        from jax.experimental.shard_map import shard_map
        from concourse.bass2jax import (_bass_exec_p, partition_id_tensor,
                                        install_neuronx_cc_hook)

        install_neuronx_cc_hook()
        self.n_cores = n_cores
        in_names, out_names, out_avals = [], [], []
        pname = nc.partition_id_tensor.name if nc.partition_id_tensor else None
        for alloc in nc.m.functions[0].allocations:
            if not isinstance(alloc, mybir.MemoryLocationSet):
                continue
            name = alloc.memorylocations[0].name
            if alloc.kind == "ExternalInput":
                if name != pname:
                    in_names.append(name)
            elif alloc.kind == "ExternalOutput":
                out_names.append(name)
                out_avals.append(jax.core.ShapedArray(
                    tuple(alloc.tensor_shape), mybir.dt.np(alloc.dtype)))
        self.in_names, self.out_names, self.out_avals = \
            in_names, out_names, out_avals
        all_in = in_names + out_names + ([pname] if pname else [])

        def _bdy(*args):
            ops = list(args)
            if pname is not None:
                ops.append(partition_id_tensor())
            return tuple(_bass_exec_p.bind(
                *ops, out_avals=tuple(out_avals), in_names=tuple(all_in),
                out_names=tuple(out_names), lowering_input_output_aliases=(),
                sim_require_finite=True, sim_require_nnan=True, nc=nc))

        mesh = Mesh(np.asarray(jax.devices()[:n_cores]), ("core",))
        nio = len(in_names) + len(out_names)
        self.sharded = jax.jit(
            shard_map(_bdy, mesh=mesh,
                      in_specs=(PartitionSpec("core"),) * nio,
                      out_specs=(PartitionSpec("core"),) * len(out_names),
                      check_rep=False),
            keep_unused=True)

    def run(self, in_maps):
        args = [np.concatenate([np.asarray(m[n]) for m in in_maps], axis=0)
                for n in self.in_names]
        args += [np.zeros((self.n_cores * a.shape[0], *a.shape[1:]), a.dtype)
                 for a in self.out_avals]
        outs = self.sharded(*args)
        return [{n: np.asarray(outs[i]).reshape(
                    self.n_cores, *self.out_avals[i].shape)[c]
                 for i, n in enumerate(self.out_names)}
                for c in range(self.n_cores)]


_EXEC_CACHE = {}


def _run_spmd(nc, in_maps, first_call):
    if not first_call:
        ex = _EXEC_CACHE.get(id(nc))
        if ex is None:
            ex = _EXEC_CACHE[id(nc)] = _SpmdExec(nc, NCORES)
        return ex.run(in_maps)
    import os
    try:
        return run_bass_kernel_spmd(
            nc, in_maps, core_ids=list(range(NCORES))).results
    except ModuleNotFoundError:
        os.environ["BASS_NEVER_TRACE"] = "1"
        return run_bass_kernel_spmd(
            nc, in_maps, core_ids=list(range(NCORES))).results


_CALLED = False


def kernel(st, W1, b1, W2, b2, W3, b3):
    global _CALLED
    st = np.ascontiguousarray(np.asarray(st, np.float32))
    in_maps = _make_in_maps(
        st,
        np.asarray(W1, np.float32), np.asarray(b1, np.float32),
        np.asarray(W2, np.float32), np.asarray(b2, np.float32),
        np.asarray(W3, np.float32))
    nc = _build_nc(1)
    results = _run_spmd(nc, in_maps, first_call=not _CALLED)
    _CALLED = True

    b3v = np.asarray(b3, np.float32).reshape(1, C)
    rows = [_rowmap(c) for c in range(C)]
    out = np.empty((B, C, 1), np.float32)
    for core in range(NCORES):
        d = results[core]["d"][rows].astype(np.float32)   # [8, 8192]
        out[core * NB:(core + 1) * NB, :, 0] = d.T + b3v
    return out
